# revision 33
# baseline (speedup 1.0000x reference)
"""MoE layer (8 experts, top-2) on 8 Trainium2 NeuronCores, expert-parallel.

Strategy:
  - Router (tiny: [8192,1024]@[1024,8]) + top-2 + combine weights run on host
    with jax-CPU, bit-matching the reference routing.
  - Tokens are gathered per expert on host; core j runs expert j's FFN
    (x @ Win^T -> silu gating -> @ Wout^T -> * combine_weight) over its
    (padded) token batch.  All heavy FLOPs (~206 GFLOP) are on device.
  - conv tap is folded into Win rows, Dp into Wout rows on host (exact).
  - Matmuls run in bf16 with fp32 PSUM accumulation; activations fp32->bf16.
  - Host scatters the per-expert outputs back (indices within one expert are
    unique, so fancy-index += is safe) and returns (out, aux_loss).
"""

import math

import numpy as np
import ml_dtypes

import concourse.bass as bass
import concourse.mybir as mybir
import concourse.tile as tile
from concourse.bass_utils import run_bass_kernel_spmd

# If tracing is requested (BASS_TRACE=1) under axon, bass_utils imports
# antenv.axon_hooks, which this image lacks.  Provide the tiny get/set module
# (with the documented ctypes NTFF hook) so a tracing run works instead of
# crashing.  Pure no-op when tracing is off.
try:
    from antenv.axon_hooks import get_axon_ntff_profile_hook  # noqa: F401
except ImportError:
    import contextlib
    import ctypes
    import sys
    import types

    _hook_holder = [None]
    _m = types.ModuleType("antenv.axon_hooks")
    _m.set_axon_ntff_profile_hook = lambda h: _hook_holder.__setitem__(0, h)
    _m.get_axon_ntff_profile_hook = lambda: _hook_holder[0]
    sys.modules["antenv.axon_hooks"] = _m

    def _ntff_profile_via_ctypes(so_path):
        try:
            lib = ctypes.CDLL(so_path)
        except OSError:
            return None
        if not hasattr(lib, "axon_start_nrt_profile"):
            return None
        lib.axon_start_nrt_profile.argtypes = [
            ctypes.POINTER(ctypes.c_int64), ctypes.c_size_t,
        ]
        lib.axon_start_nrt_profile.restype = ctypes.c_int64
        lib.axon_stop_nrt_profile.argtypes = [ctypes.c_char_p]
        lib.axon_stop_nrt_profile.restype = ctypes.c_int64

        @contextlib.contextmanager
        def _hook_cm(output_dir, device_ids):
            import jax

            jax.devices()
            if device_ids:
                ids = (ctypes.c_int64 * len(device_ids))(*device_ids)
                rc = lib.axon_start_nrt_profile(ids, len(device_ids))
            else:
                rc = lib.axon_start_nrt_profile(None, 0)
            if rc != 0:
                raise RuntimeError(f"axon_start_nrt_profile rc={rc}")
            try:
                yield
            finally:
                n = lib.axon_stop_nrt_profile(str(output_dir).encode())
                if n < 0:
                    raise RuntimeError(f"axon_stop_nrt_profile rc={n}")

        return _hook_cm

    _m.set_axon_ntff_profile_hook(
        _ntff_profile_via_ctypes("/opt/axon/libaxon_pjrt.so")
    )

D_MODEL = 1024
D_INNER = 2048
N_EXPERTS = 8
TOP_K = 2
N_CORES = 8
CT = D_MODEL // 128   # 8 contraction tiles for matmul1
IT = D_INNER // 128   # 16 contraction tiles for matmul2
DT = D_MODEL // 128   # 8 output tiles

BF16 = mybir.dt.bfloat16
F32 = mybir.dt.float32

# ---------------------------------------------------------------------------
# Workaround: this walrus build rejects >1 sync-wait per instruction
# ("Too many sync wait commands" in setupSyncWait).  After Tile scheduling,
# split every multi-wait instruction: excess waits move to single-wait NoOps
# inserted right before it on the same engine (semantically identical — the
# engine executes its queue in order, so all waits still precede the op).
_MAX_WAITS = 1


def _split_multi_waits(nc):
    n_new = 0
    for f in nc.m.functions:
        for b in f.blocks:
            insts = list(b.instructions)
            need = any(
                i.sync_info and i.sync_info.on_wait
                and len(i.sync_info.on_wait) > _MAX_WAITS
                for i in insts
            )
            if not need:
                continue
            new = []
            for inst in insts:
                si = inst.sync_info
                if si is not None and si.on_wait and len(si.on_wait) > _MAX_WAITS:
                    waits = list(si.on_wait)
                    for w in waits[:-_MAX_WAITS]:
                        n = mybir.InstNoOp(
                            name=f"{inst.name}_sw{n_new}", ins=[], outs=[]
                        )
                        n.engine = inst.engine
                        n.sync_info = mybir.SyncInfo(on_wait=[w], on_update=[])
                        new.append(n)
                        n_new += 1
                    si.on_wait = waits[-_MAX_WAITS:]
                new.append(inst)
            b.instructions = new
    return n_new
# ---------------------------------------------------------------------------

_last_run = None        # BassKernelResults of the most recent launch (for test harness)
_program_cache = {}     # cap -> compiled Bass program


def _chunk_plan(cap):
    """Split cap (multiple of 32) into chunks <=512, each a multiple of 32,
    as equal as possible (all >=256 when cap allows, so LDWEIGHTS hides)."""
    units = cap // 32
    n_chunks = math.ceil(units / 16)
    base = units // n_chunks
    rem = units - base * n_chunks
    return [32 * (base + (1 if i < rem else 0)) for i in range(n_chunks)]


def _build_program(cap):
    chunks = _chunk_plan(cap)
    nc = bass.Bass()
    # w1k[k] packs the two 128-col stationary blocks (xi | z) for inner tile k
    # across all CT contraction tiles, so one DMA delivers one k-iteration's
    # weights and the first matmul only waits on ~0.9MB, not all of w1.
    w1k = nc.dram_tensor("w1k", [IT, 128, CT, 256], BF16, kind="ExternalInput")
    w2 = nc.dram_tensor("w2", [128, IT, D_MODEL], BF16, kind="ExternalInput")
    xg = nc.dram_tensor("xg", [128, CT, cap], BF16, kind="ExternalInput")
    cb = nc.dram_tensor("cb", [128, IT], F32, kind="ExternalInput")
    cwb = nc.dram_tensor("cwb", [128, cap], F32, kind="ExternalInput")
    out = nc.dram_tensor("out", [128, DT, cap], F32, kind="ExternalOutput")

    with tile.TileContext(nc) as tc:
        with (
            tc.tile_pool(name="wpool", bufs=1) as wpool,
            tc.tile_pool(name="xpool", bufs=3) as xpool,
            tc.tile_pool(name="ypool", bufs=2) as ypool,
            tc.tile_pool(name="apool", bufs=4) as apool,
            tc.tile_pool(name="opool", bufs=3) as opool,
            tc.tile_pool(name="px", bufs=3, space="PSUM") as pxpool,
            tc.tile_pool(name="pz", bufs=3, space="PSUM") as pzpool,
            tc.tile_pool(name="po", bufs=2, space="PSUM") as popool,
        ):
            # PE warmup: dummy matmuls on scratch SBUF while the first DMAs
            # land, so HAM is at 2.4GHz when the real stream begins.  The
            # results land in a scratch PSUM bank and are never read.
            warm_sb = wpool.tile([128, 512], BF16, tag="warm")
            nc.vector.memset(warm_sb[:], 0.0)
            warm_ps = popool.tile([128, 512], F32, tag="po")
            for _ in range(8):
                nc.tensor.matmul(
                    warm_ps[:], warm_sb[:, 0:128], warm_sb[:],
                    start=True, stop=True,
                )

            # chunk 0's tokens and k=0 weights split into half-c tiles so the
            # first matmuls wait on ~0.4MB of DMA, not the full 1.35MB
            hc = CT // 2
            x0a = xpool.tile([128, hc, chunks[0]], BF16, tag="x0a")
            x0b = xpool.tile([128, CT - hc, chunks[0]], BF16, tag="x0b")
            w1k0a = wpool.tile([128, hc, 256], BF16, tag="w1k0a")
            w1k0b = wpool.tile([128, CT - hc, 256], BF16, tag="w1k0b")
            nc.sync.dma_start(x0a[:], xg[:, 0:hc, 0:chunks[0]])
            nc.sync.dma_start(w1k0a[:], w1k[0, :, 0:hc, :])
            nc.sync.dma_start(x0b[:], xg[:, hc:CT, 0:chunks[0]])
            nc.sync.dma_start(w1k0b[:], w1k[0, :, hc:CT, :])
            cb_sb = wpool.tile([128, IT], F32, tag="cb")
            nc.sync.dma_start(cb_sb[:], cb[:])
            w1k_sb = [None]
            for k in range(1, IT):
                w1k_tile = wpool.tile([128, CT, 256], BF16, tag=f"w1k{k}")
                nc.sync.dma_start(w1k_tile[:], w1k[k])
                w1k_sb.append(w1k_tile)
            w2_sb = wpool.tile([128, IT, D_MODEL], BF16, tag="w2")
            nc.sync.dma_start(w2_sb[:], w2[:])
            cwb_sb = wpool.tile([128, cap], F32, tag="cwb")
            nc.sync.dma_start(cwb_sb[:], cwb[:])

            n0 = 0
            for ci, ns in enumerate(chunks):
                if ci == 0:
                    x_at = lambda c: x0a[:, c, :] if c < hc else x0b[:, c - hc, :]
                else:
                    x_sb = xpool.tile([128, CT, ns], BF16, tag="x")
                    nc.sync.dma_start(x_sb[:], xg[:, :, n0:n0 + ns])
                    x_at = lambda c, t=x_sb: t[:, c, :]

                def w1_at(k, c, lo, hi):
                    if k == 0:
                        t = w1k0a if c < hc else w1k0b
                        return t[:, c if c < hc else c - hc, lo:hi]
                    return w1k_sb[k][:, c, lo:hi]

                y_sb = ypool.tile([128, IT, ns], BF16, tag="y")
                for k in range(IT):
                    px = pxpool.tile([128, ns], F32, tag="px")
                    pz = pzpool.tile([128, ns], F32, tag="pz")
                    for c in range(CT):
                        nc.tensor.matmul(
                            px[:],
                            w1_at(k, c, 0, 128),
                            x_at(c),
                            start=(c == 0),
                            stop=(c == CT - 1),
                        )
                    for c in range(CT):
                        nc.tensor.matmul(
                            pz[:],
                            w1_at(k, c, 128, 256),
                            x_at(c),
                            start=(c == 0),
                            stop=(c == CT - 1),
                        )
                    xa = apool.tile([128, ns], BF16, tag="xa")
                    sz = apool.tile([128, ns], BF16, tag="sz")
                    nc.scalar.activation(
                        xa[:], px[:], mybir.ActivationFunctionType.Silu,
                        bias=cb_sb[:, k:k + 1],
                    )
                    nc.scalar.activation(
                        sz[:], pz[:], mybir.ActivationFunctionType.Silu,
                    )
                    nc.vector.tensor_mul(y_sb[:, k, :], xa[:], sz[:])
                for d in range(DT):
                    po = popool.tile([128, ns], F32, tag="po")
                    for it in range(IT):
                        nc.tensor.matmul(
                            po[:],
                            w2_sb[:, it, d * 128:(d + 1) * 128],
                            y_sb[:, it, :],
                            start=(it == 0),
                            stop=(it == IT - 1),
                        )
                    o_sb = opool.tile([128, ns], F32, tag="o")
                    nc.vector.tensor_mul(o_sb[:], po[:], cwb_sb[:, n0:n0 + ns])
                    nc.sync.dma_start(out[:, d, n0:n0 + ns], o_sb[:])
                n0 += ns
    _split_multi_waits(nc)
    return nc


def _routing(xf, Wr, br):
    """Reference-identical routing on jax CPU. Returns (cw [N,E] f32, aux f32)."""
    import jax
    import jax.numpy as jnp

    cpu = jax.devices("cpu")[0]
    with jax.default_device(cpu):
        logits = jnp.asarray(xf) @ jnp.asarray(Wr).T + jnp.asarray(br)
        probs = jax.nn.softmax(logits.astype(jnp.float32), axis=-1)
        tw, ti = jax.lax.top_k(probs, TOP_K)
        tw = tw / jnp.sum(tw, axis=-1, keepdims=True)
        oh = jax.nn.one_hot(ti, N_EXPERTS, dtype=jnp.float32)
        load = oh.sum(1).mean(0)
        aux_loss = jnp.sum(load * load)
        cw = jnp.einsum('nk,nke->ne', tw, oh)
    return np.asarray(cw), np.asarray(aux_loss), np.asarray(ti)


def kernel(x, Wr, br, Win, conv_w, conv_b, Dp, Wout):
    global _last_run
    x = np.asarray(x, np.float32)
    Wr = np.asarray(Wr, np.float32)
    br = np.asarray(br, np.float32)
    Win = np.asarray(Win, np.float32)
    conv_w = np.asarray(conv_w, np.float32)
    conv_b = np.asarray(conv_b, np.float32)
    Dp = np.asarray(Dp, np.float32)
    Wout = np.asarray(Wout, np.float32)

    B, L, C = x.shape
    N = B * L
    xf = x.reshape(N, C)

    cw, aux_loss, _ti = _routing(xf, Wr, br)

    idxs = [np.nonzero(cw[:, j] > 0)[0] for j in range(N_EXPERTS)]
    counts = [len(i) for i in idxs]
    cap = max(512, 32 * math.ceil(max(counts) / 32))

    if cap not in _program_cache:
        _program_cache[cap] = _build_program(cap)
    nc = _program_cache[cap]

    xfT = xf.T  # [C, N]
    in_maps = []
    for j in range(N_EXPERTS):
        idx = idxs[j]
        nj = counts[j]
        # tokens, feature-major: xg[p, ct, n] = x[token n, ct*128+p]
        xg = np.zeros((128, CT, cap), ml_dtypes.bfloat16)
        xg[:, :, :nj] = (
            xfT[:, idx].astype(ml_dtypes.bfloat16)
            .reshape(CT, 128, nj).transpose(1, 0, 2)
        )
        # Win with conv tap folded into the xi half rows
        w1eff = Win[j].copy()
        w1eff[:D_INNER] *= conv_w[j, :, -1][:, None]
        # [2*D_INNER, C] -> per-k blocks: w1k[k, p, c, 0:128]=xi, [128:256]=z
        w1t = w1eff.T.reshape(CT, 128, 2 * D_INNER)   # [c_t, p, m]
        w1knp = np.empty((IT, 128, CT, 256), np.float32)
        for k in range(IT):
            w1knp[k, :, :, 0:128] = w1t[:, :, k * 128:(k + 1) * 128].transpose(1, 0, 2)
            w1knp[k, :, :, 128:256] = w1t[
                :, :, D_INNER + k * 128:D_INNER + (k + 1) * 128
            ].transpose(1, 0, 2)
        w1 = w1knp.astype(ml_dtypes.bfloat16)
        # Wout with Dp folded into inner-dim columns
        w2eff = Wout[j] * Dp[j][None, :]
        w2 = np.ascontiguousarray(
            w2eff.T.reshape(IT, 128, D_MODEL).transpose(1, 0, 2)
        ).astype(ml_dtypes.bfloat16)
        cb = np.ascontiguousarray(conv_b[j].reshape(IT, 128).T)
        cwj = np.zeros((cap,), np.float32)
        cwj[:nj] = cw[idx, j]
        cwb = np.ascontiguousarray(np.broadcast_to(cwj[None, :], (128, cap)))
        in_maps.append({"xg": xg, "w1k": w1, "w2": w2, "cb": cb, "cwb": cwb})

    res = run_bass_kernel_spmd(nc, in_maps, list(range(N_CORES)))
    _last_run = res

    out = np.zeros((N, C), np.float32)
    for j in range(N_EXPERTS):
        idx = idxs[j]
        nj = counts[j]
        oj = res.results[j]["out"]  # [128, DT, cap] f32
        yj = oj.transpose(1, 0, 2).reshape(D_MODEL, cap)[:, :nj]
        out[idx] += yj.T
    return out.reshape(B, L, C), aux_loss


# revision 34
# speedup vs baseline: 1.0025x; 1.0025x over previous
"""MoE layer (8 experts, top-2) on 8 Trainium2 NeuronCores, expert-parallel.

Strategy:
  - Router (tiny: [8192,1024]@[1024,8]) + top-2 + combine weights run on host
    with jax-CPU, bit-matching the reference routing.
  - Tokens are gathered per expert on host; core j runs expert j's FFN
    (x @ Win^T -> silu gating -> @ Wout^T -> * combine_weight) over its
    (padded) token batch.  All heavy FLOPs (~206 GFLOP) are on device.
  - conv tap is folded into Win rows, Dp into Wout rows on host (exact).
  - Matmuls run in bf16 with fp32 PSUM accumulation; activations fp32->bf16.
  - Host scatters the per-expert outputs back (indices within one expert are
    unique, so fancy-index += is safe) and returns (out, aux_loss).
"""

import math

import numpy as np
import ml_dtypes

import concourse.bass as bass
import concourse.mybir as mybir
import concourse.tile as tile
from concourse.bass_utils import run_bass_kernel_spmd

# If tracing is requested (BASS_TRACE=1) under axon, bass_utils imports
# antenv.axon_hooks, which this image lacks.  Provide the tiny get/set module
# (with the documented ctypes NTFF hook) so a tracing run works instead of
# crashing.  Pure no-op when tracing is off.
try:
    from antenv.axon_hooks import get_axon_ntff_profile_hook  # noqa: F401
except ImportError:
    import contextlib
    import ctypes
    import sys
    import types

    _hook_holder = [None]
    _m = types.ModuleType("antenv.axon_hooks")
    _m.set_axon_ntff_profile_hook = lambda h: _hook_holder.__setitem__(0, h)
    _m.get_axon_ntff_profile_hook = lambda: _hook_holder[0]
    sys.modules["antenv.axon_hooks"] = _m

    def _ntff_profile_via_ctypes(so_path):
        try:
            lib = ctypes.CDLL(so_path)
        except OSError:
            return None
        if not hasattr(lib, "axon_start_nrt_profile"):
            return None
        lib.axon_start_nrt_profile.argtypes = [
            ctypes.POINTER(ctypes.c_int64), ctypes.c_size_t,
        ]
        lib.axon_start_nrt_profile.restype = ctypes.c_int64
        lib.axon_stop_nrt_profile.argtypes = [ctypes.c_char_p]
        lib.axon_stop_nrt_profile.restype = ctypes.c_int64

        @contextlib.contextmanager
        def _hook_cm(output_dir, device_ids):
            import jax

            jax.devices()
            if device_ids:
                ids = (ctypes.c_int64 * len(device_ids))(*device_ids)
                rc = lib.axon_start_nrt_profile(ids, len(device_ids))
            else:
                rc = lib.axon_start_nrt_profile(None, 0)
            if rc != 0:
                raise RuntimeError(f"axon_start_nrt_profile rc={rc}")
            try:
                yield
            finally:
                n = lib.axon_stop_nrt_profile(str(output_dir).encode())
                if n < 0:
                    raise RuntimeError(f"axon_stop_nrt_profile rc={n}")

        return _hook_cm

    _m.set_axon_ntff_profile_hook(
        _ntff_profile_via_ctypes("/opt/axon/libaxon_pjrt.so")
    )

D_MODEL = 1024
D_INNER = 2048
N_EXPERTS = 8
TOP_K = 2
N_CORES = 8
CT = D_MODEL // 128   # 8 contraction tiles for matmul1
IT = D_INNER // 128   # 16 contraction tiles for matmul2
DT = D_MODEL // 128   # 8 output tiles

BF16 = mybir.dt.bfloat16
F32 = mybir.dt.float32

# ---------------------------------------------------------------------------
# Workaround: this walrus build rejects >1 sync-wait per instruction
# ("Too many sync wait commands" in setupSyncWait).  After Tile scheduling,
# split every multi-wait instruction: excess waits move to single-wait NoOps
# inserted right before it on the same engine (semantically identical — the
# engine executes its queue in order, so all waits still precede the op).
_MAX_WAITS = 1


def _split_multi_waits(nc):
    n_new = 0
    for f in nc.m.functions:
        for b in f.blocks:
            insts = list(b.instructions)
            need = any(
                i.sync_info and i.sync_info.on_wait
                and len(i.sync_info.on_wait) > _MAX_WAITS
                for i in insts
            )
            if not need:
                continue
            new = []
            for inst in insts:
                si = inst.sync_info
                if si is not None and si.on_wait and len(si.on_wait) > _MAX_WAITS:
                    waits = list(si.on_wait)
                    for w in waits[:-_MAX_WAITS]:
                        n = mybir.InstNoOp(
                            name=f"{inst.name}_sw{n_new}", ins=[], outs=[]
                        )
                        n.engine = inst.engine
                        n.sync_info = mybir.SyncInfo(on_wait=[w], on_update=[])
                        new.append(n)
                        n_new += 1
                    si.on_wait = waits[-_MAX_WAITS:]
                new.append(inst)
            b.instructions = new
    return n_new
# ---------------------------------------------------------------------------

_last_run = None        # BassKernelResults of the most recent launch (for test harness)
_program_cache = {}     # cap -> compiled Bass program


def _chunk_plan(cap):
    """Split cap (multiple of 32) into chunks <=512, each a multiple of 32,
    as equal as possible (all >=256 when cap allows, so LDWEIGHTS hides)."""
    units = cap // 32
    n_chunks = math.ceil(units / 16)
    base = units // n_chunks
    rem = units - base * n_chunks
    return [32 * (base + (1 if i < rem else 0)) for i in range(n_chunks)]


def _build_program(cap):
    chunks = _chunk_plan(cap)
    nc = bass.Bass()
    # w1k[k] packs the two 128-col stationary blocks (xi | z) for inner tile k
    # across all CT contraction tiles, so one DMA delivers one k-iteration's
    # weights and the first matmul only waits on ~0.9MB, not all of w1.
    w1k = nc.dram_tensor("w1k", [IT, 128, CT, 256], BF16, kind="ExternalInput")
    w2 = nc.dram_tensor("w2", [128, IT, D_MODEL], BF16, kind="ExternalInput")
    xg = nc.dram_tensor("xg", [128, CT, cap], BF16, kind="ExternalInput")
    cb = nc.dram_tensor("cb", [128, IT], F32, kind="ExternalInput")
    cwb = nc.dram_tensor("cwb", [128, cap], F32, kind="ExternalInput")
    out = nc.dram_tensor("out", [128, DT, cap], F32, kind="ExternalOutput")

    with tile.TileContext(nc) as tc:
        with (
            tc.tile_pool(name="wpool", bufs=1) as wpool,
            tc.tile_pool(name="xpool", bufs=3) as xpool,
            tc.tile_pool(name="ypool", bufs=2) as ypool,
            tc.tile_pool(name="apool", bufs=4) as apool,
            tc.tile_pool(name="opool", bufs=3) as opool,
            tc.tile_pool(name="px", bufs=3, space="PSUM") as pxpool,
            tc.tile_pool(name="pz", bufs=3, space="PSUM") as pzpool,
            tc.tile_pool(name="po", bufs=2, space="PSUM") as popool,
        ):
            # PE warmup: dummy matmuls on scratch SBUF while the first DMAs
            # land, so HAM is at 2.4GHz when the real stream begins.  The
            # results land in a scratch PSUM bank and are never read.
            warm_sb = wpool.tile([128, 512], BF16, tag="warm")
            nc.vector.memset(warm_sb[:], 0.0)
            warm_ps = popool.tile([128, 512], F32, tag="po")
            for _ in range(13):
                nc.tensor.matmul(
                    warm_ps[:], warm_sb[:, 0:128], warm_sb[:],
                    start=True, stop=True,
                )

            x0_sb = xpool.tile([128, CT, chunks[0]], BF16, tag="x")
            nc.sync.dma_start(x0_sb[:], xg[:, :, 0:chunks[0]])
            cb_sb = wpool.tile([128, IT], F32, tag="cb")
            w1k_sb = []
            for k in range(IT):
                w1k_tile = wpool.tile([128, CT, 256], BF16, tag=f"w1k{k}")
                nc.sync.dma_start(w1k_tile[:], w1k[k])
                w1k_sb.append(w1k_tile)
                if k == 0:
                    nc.sync.dma_start(cb_sb[:], cb[:])
            w2_sb = wpool.tile([128, IT, D_MODEL], BF16, tag="w2")
            nc.sync.dma_start(w2_sb[:], w2[:])
            cwb_sb = wpool.tile([128, cap], F32, tag="cwb")
            nc.sync.dma_start(cwb_sb[:], cwb[:])

            n0 = 0
            for ci, ns in enumerate(chunks):
                if ci == 0:
                    x_sb = x0_sb
                else:
                    x_sb = xpool.tile([128, CT, ns], BF16, tag="x")
                    nc.sync.dma_start(x_sb[:], xg[:, :, n0:n0 + ns])
                x_at = lambda c, t=x_sb: t[:, c, :]

                def w1_at(k, c, lo, hi):
                    return w1k_sb[k][:, c, lo:hi]

                y_sb = ypool.tile([128, IT, ns], BF16, tag="y")
                for k in range(IT):
                    px = pxpool.tile([128, ns], F32, tag="px")
                    pz = pzpool.tile([128, ns], F32, tag="pz")
                    for c in range(CT):
                        nc.tensor.matmul(
                            px[:],
                            w1_at(k, c, 0, 128),
                            x_at(c),
                            start=(c == 0),
                            stop=(c == CT - 1),
                        )
                    for c in range(CT):
                        nc.tensor.matmul(
                            pz[:],
                            w1_at(k, c, 128, 256),
                            x_at(c),
                            start=(c == 0),
                            stop=(c == CT - 1),
                        )
                    xa = apool.tile([128, ns], BF16, tag="xa")
                    sz = apool.tile([128, ns], BF16, tag="sz")
                    nc.scalar.activation(
                        xa[:], px[:], mybir.ActivationFunctionType.Silu,
                        bias=cb_sb[:, k:k + 1],
                    )
                    nc.scalar.activation(
                        sz[:], pz[:], mybir.ActivationFunctionType.Silu,
                    )
                    nc.vector.tensor_mul(y_sb[:, k, :], xa[:], sz[:])
                for d in range(DT):
                    po = popool.tile([128, ns], F32, tag="po")
                    for it in range(IT):
                        nc.tensor.matmul(
                            po[:],
                            w2_sb[:, it, d * 128:(d + 1) * 128],
                            y_sb[:, it, :],
                            start=(it == 0),
                            stop=(it == IT - 1),
                        )
                    o_sb = opool.tile([128, ns], F32, tag="o")
                    nc.vector.tensor_mul(o_sb[:], po[:], cwb_sb[:, n0:n0 + ns])
                    nc.sync.dma_start(out[:, d, n0:n0 + ns], o_sb[:])
                n0 += ns
    _split_multi_waits(nc)
    return nc


def _routing(xf, Wr, br):
    """Reference-identical routing on jax CPU. Returns (cw [N,E] f32, aux f32)."""
    import jax
    import jax.numpy as jnp

    cpu = jax.devices("cpu")[0]
    with jax.default_device(cpu):
        logits = jnp.asarray(xf) @ jnp.asarray(Wr).T + jnp.asarray(br)
        probs = jax.nn.softmax(logits.astype(jnp.float32), axis=-1)
        tw, ti = jax.lax.top_k(probs, TOP_K)
        tw = tw / jnp.sum(tw, axis=-1, keepdims=True)
        oh = jax.nn.one_hot(ti, N_EXPERTS, dtype=jnp.float32)
        load = oh.sum(1).mean(0)
        aux_loss = jnp.sum(load * load)
        cw = jnp.einsum('nk,nke->ne', tw, oh)
    return np.asarray(cw), np.asarray(aux_loss), np.asarray(ti)


def kernel(x, Wr, br, Win, conv_w, conv_b, Dp, Wout):
    global _last_run
    x = np.asarray(x, np.float32)
    Wr = np.asarray(Wr, np.float32)
    br = np.asarray(br, np.float32)
    Win = np.asarray(Win, np.float32)
    conv_w = np.asarray(conv_w, np.float32)
    conv_b = np.asarray(conv_b, np.float32)
    Dp = np.asarray(Dp, np.float32)
    Wout = np.asarray(Wout, np.float32)

    B, L, C = x.shape
    N = B * L
    xf = x.reshape(N, C)

    cw, aux_loss, _ti = _routing(xf, Wr, br)

    idxs = [np.nonzero(cw[:, j] > 0)[0] for j in range(N_EXPERTS)]
    counts = [len(i) for i in idxs]
    cap = max(512, 32 * math.ceil(max(counts) / 32))

    if cap not in _program_cache:
        _program_cache[cap] = _build_program(cap)
    nc = _program_cache[cap]

    xfT = xf.T  # [C, N]
    in_maps = []
    for j in range(N_EXPERTS):
        idx = idxs[j]
        nj = counts[j]
        # tokens, feature-major: xg[p, ct, n] = x[token n, ct*128+p]
        xg = np.zeros((128, CT, cap), ml_dtypes.bfloat16)
        xg[:, :, :nj] = (
            xfT[:, idx].astype(ml_dtypes.bfloat16)
            .reshape(CT, 128, nj).transpose(1, 0, 2)
        )
        # Win with conv tap folded into the xi half rows
        w1eff = Win[j].copy()
        w1eff[:D_INNER] *= conv_w[j, :, -1][:, None]
        # [2*D_INNER, C] -> per-k blocks: w1k[k, p, c, 0:128]=xi, [128:256]=z
        w1t = w1eff.T.reshape(CT, 128, 2 * D_INNER)   # [c_t, p, m]
        w1knp = np.empty((IT, 128, CT, 256), np.float32)
        for k in range(IT):
            w1knp[k, :, :, 0:128] = w1t[:, :, k * 128:(k + 1) * 128].transpose(1, 0, 2)
            w1knp[k, :, :, 128:256] = w1t[
                :, :, D_INNER + k * 128:D_INNER + (k + 1) * 128
            ].transpose(1, 0, 2)
        w1 = w1knp.astype(ml_dtypes.bfloat16)
        # Wout with Dp folded into inner-dim columns
        w2eff = Wout[j] * Dp[j][None, :]
        w2 = np.ascontiguousarray(
            w2eff.T.reshape(IT, 128, D_MODEL).transpose(1, 0, 2)
        ).astype(ml_dtypes.bfloat16)
        cb = np.ascontiguousarray(conv_b[j].reshape(IT, 128).T)
        cwj = np.zeros((cap,), np.float32)
        cwj[:nj] = cw[idx, j]
        cwb = np.ascontiguousarray(np.broadcast_to(cwj[None, :], (128, cap)))
        in_maps.append({"xg": xg, "w1k": w1, "w2": w2, "cb": cb, "cwb": cwb})

    res = run_bass_kernel_spmd(nc, in_maps, list(range(N_CORES)))
    _last_run = res

    out = np.zeros((N, C), np.float32)
    for j in range(N_EXPERTS):
        idx = idxs[j]
        nj = counts[j]
        oj = res.results[j]["out"]  # [128, DT, cap] f32
        yj = oj.transpose(1, 0, 2).reshape(D_MODEL, cap)[:, :nj]
        out[idx] += yj.T
    return out.reshape(B, L, C), aux_loss
